# revision 2
# baseline (speedup 1.0000x reference)
"""DenseTopKSAE kernel v4 for Trainium2 (8 NeuronCores, Bass/Tile).

Sharding: expert-parallel over R -- core r owns SAE r. No collectives.

Structure (per core):
  X: x loaded, db subtracted (ones-matmul broadcast), fp16 hi/lo split,
     staged to DRAM per b-half, dense xbar transpose-load -> xcT halves.
     The encoder-weight chain for block 0 is issued first so both
     startup chains overlap.
  E: per 512-d block (software-pipelined one block ahead): ew fp32 load,
     64-scaled fp16 hi/lo split (ACT + one fused DVE op), staged to DRAM
     [d, hi|lo], ONE xbar transpose-load -> ewT [128c, 16, 512]; eb
     broadcast via K=1 ones-matmuls; 24 fp16 MMs per b-tile (3-term
     split, fp32-exact); +eb, un-scale, h -> DRAM fp32; DVE max8
     candidates per 256-chunk.
  T: max8 + match_replace rounds on candidates -> k-th largest t per row.
  D: per 1024-d block, loads prefetched two blocks ahead: dw loaded with
     SWDGE cast-DMA directly to fp16; h reloaded; mask = (h>=t)*h as
     DVE is_ge (stride-0 broadcast of t) + gpsimd multiply; dw/hm
     PE-transposed (fp16, cheap) right before the block's MMs; 8 MMs per
     (b-tile, c-half) accumulate 1024 d; db bias via ones-MMs on block
     0; output accumulated in SBUF (DVE adds), stored at the end.
"""

import numpy as np

import concourse.bass as bass
import concourse.mybir as mybir
import concourse.tile as tile
from concourse import bacc
from concourse.bass_utils import run_bass_kernel_spmd

F32 = mybir.dt.float32
F16 = mybir.dt.float16
AF = mybir.ActivationFunctionType
ALU = mybir.AluOpType
P = 128
NEG = -3.0e38
WSCALE = 64.0

B, R, C, D = 1024, 8, 1024, 16384
N_CORES = 8

DBLK = 512      # encode d-block
DBLK2 = 1024    # decode d-block
CHUNK = 256     # candidate chunk (top-8 per chunk must cover top-k)


def _mk_identity(nc, ident, fill):
    nc.gpsimd.memset(ident, 0.0)
    nc.gpsimd.affine_select(
        out=ident, in_=ident, compare_op=ALU.not_equal, fill=fill,
        base=0, pattern=[[-1, ident.shape[0]]], channel_multiplier=1,
    )


def _encode_prep(nc, encp, ebps, ew_d, ew16_d, eb_d, dblk, nct, c):
    """Stage + transpose-load one 512-d block of encoder weight + bias."""
    na = DBLK // P
    d0 = dblk * DBLK
    dsl = slice(d0, d0 + DBLK)
    ewn = encp.tile([P, na, c], F32, tag="ewn")
    nc.sync.dma_start(
        out=ewn, in_=ew_d[dsl, :].rearrange("(a p) c -> p a c", p=P))
    wh = encp.tile([P, na, c], F16, tag="wh")
    nc.scalar.activation(wh, ewn, AF.Copy, scale=WSCALE)
    wl = encp.tile([P, na, c], F16, tag="wl")
    nc.vector.scalar_tensor_tensor(
        out=wl, in0=ewn, scalar=WSCALE, in1=wh,
        op0=ALU.mult, op1=ALU.subtract)
    nc.sync.dma_start(
        out=ew16_d[dsl, 0:c].rearrange("(a p) c -> p a c", p=P), in_=wh)
    nc.sync.dma_start(
        out=ew16_d[dsl, c:2 * c].rearrange("(a p) c -> p a c", p=P), in_=wl)
    ewT = encp.tile([P, 2 * nct, DBLK], F16, tag="ewT")
    nc.sync.dma_start(out=ewT, in_=ew16_d[dsl, :], transpose=True)
    ebs = encp.tile([1, DBLK], F32, tag="ebs")
    nc.sync.dma_start(out=ebs,
                      in_=eb_d[dsl].rearrange("(a n) -> a n", a=1))
    ebh = encp.tile([1, DBLK], F16, tag="ebh")
    nc.vector.tensor_copy(ebh, ebs)
    ebl = encp.tile([1, DBLK], F16, tag="ebl")
    nc.vector.tensor_sub(ebl, ebs, ebh)
    pe_b = ebps.tile([P, DBLK], F32, tag="ebps")
    nc.tensor.matmul(pe_b, ones_g[0], ebh, start=True, stop=False)
    nc.tensor.matmul(pe_b, ones_g[0], ebl, start=False, stop=True)
    eb64 = encp.tile([P, DBLK], F32, tag="eb64")
    nc.scalar.activation(eb64, pe_b, AF.Copy, scale=WSCALE)
    return ewT, eb64


ones_g = [None]


def build(k, b=B, c=C, d=D):
    nb, nct = b // P, c // P
    ndblk = d // DBLK
    nd2 = d // DBLK2
    nch = d // CHUNK
    nh = nb // 2

    nc = bacc.Bacc("TRN2", target_bir_lowering=False, debug=False,
                   num_devices=N_CORES)
    x_d = nc.declare_dram_parameter("x", [b, c], F32, isOutput=False)
    ew_d = nc.declare_dram_parameter("encoder_w", [d, c], F32, isOutput=False)
    eb_d = nc.declare_dram_parameter("encoder_b", [d], F32, isOutput=False)
    dw_d = nc.declare_dram_parameter("decoder_w", [c, d], F32, isOutput=False)
    db_d = nc.declare_dram_parameter("decoder_b", [c], F32, isOutput=False)
    out_d = nc.declare_dram_parameter("out", [b, c], F32, isOutput=True)
    h_d = nc.dram_tensor("h_scratch", [b, d], F32)
    ew16_d = nc.dram_tensor("ew16_scratch", [d, 2 * c], F16)
    x16_h = [nc.dram_tensor(f"x16_scratch{i}", [b // 2, 2 * c], F16)
             for i in range(2)]

    with tile.TileContext(nc) as tc:
        with tc.tile_pool(name="persist", bufs=1) as pp:
            ones16 = pp.tile([1, P], F16, tag="ones16")
            nc.vector.memset(ones16, 1.0)
            ones_g[0] = ones16
            ident16 = pp.tile([P, P], F16, tag="ident16")
            _mk_identity(nc, ident16, 1.0)

            db_hi = pp.tile([1, c], F16, tag="dbhi")
            db_lo = pp.tile([1, c], F16, tag="dblo")
            with tc.tile_pool(name="bprep", bufs=1) as bp:
                db_row = bp.tile([1, c], F32, tag="dbrow")
                nc.sync.dma_start(out=db_row,
                                  in_=db_d.rearrange("(a n) -> a n", a=1))
                nc.vector.tensor_copy(db_hi, db_row)
                nc.vector.tensor_sub(db_lo, db_row, db_hi)

            t_sb = pp.tile([P, nb], F32, tag="tsb")

            with tc.tile_pool(name="candp", bufs=1) as cp:
                cand = [cp.tile([P, nch * 8], F32, tag=f"cand{bt}",
                                name=f"cand{bt}") for bt in range(nb)]
                with (
                    tc.tile_pool(name="xcpool", bufs=1) as xcp,
                    tc.tile_pool(name="enc", bufs=2) as encp,
                    tc.tile_pool(name="encs", bufs=3) as encs,
                    tc.tile_pool(name="ph0", bufs=2) as ph0,
                    tc.tile_pool(name="encps", bufs=3, space="PSUM") as encps,
                    tc.tile_pool(name="ebps", bufs=2, space="PSUM") as ebps,
                ):
                    xcT_h = [xcp.tile([P, 2 * nct, b // 2], F16,
                                      tag=f"xcT{i}", name=f"xcT{i}")
                             for i in range(2)]
                    # encoder block 0 chain first (overlaps the x phase)
                    enc_tiles = _encode_prep(nc, encp, ebps, ew_d, ew16_d,
                                             eb_d, 0, nct, c)
                    # db broadcast for the x subtraction
                    db_bc = ph0.tile([P, c], F32, tag="dbbc", bufs=1)
                    pb = ebps.tile([P, c], F32, tag="dbbc_ps", bufs=1)
                    nc.tensor.matmul(pb[:, 0:512], ones16, db_hi[:, 0:512],
                                     start=True, stop=False)
                    nc.tensor.matmul(pb[:, 0:512], ones16, db_lo[:, 0:512],
                                     start=False, stop=True)
                    nc.tensor.matmul(pb[:, 512:c], ones16, db_hi[:, 512:c],
                                     start=True, stop=False)
                    nc.tensor.matmul(pb[:, 512:c], ones16, db_lo[:, 512:c],
                                     start=False, stop=True)
                    nc.scalar.activation(db_bc, pb, AF.Copy)
                    # x phase: split + stage per b-half
                    for bt in range(nb):
                        xt = ph0.tile([P, c], F32, tag="xstage")
                        nc.sync.dma_start(out=xt,
                                          in_=x_d[bt * P:(bt + 1) * P, :])
                        nc.vector.tensor_sub(xt, xt, db_bc)
                        xh = ph0.tile([P, c], F16, tag="xh")
                        nc.scalar.activation(xh, xt, AF.Copy)
                        xl = ph0.tile([P, c], F16, tag="xl")
                        nc.vector.tensor_sub(xl, xt, xh)
                        half, hb = divmod(bt, nh)
                        hsl = slice(hb * P, (hb + 1) * P)
                        nc.sync.dma_start(out=x16_h[half][hsl, 0:c], in_=xh)
                        nc.sync.dma_start(out=x16_h[half][hsl, c:2 * c],
                                          in_=xl)
                        if hb == nh - 1:
                            nc.sync.dma_start(out=xcT_h[half],
                                              in_=x16_h[half][:, :],
                                              transpose=True)
                    # encode main loop, one-block software pipeline
                    for dblk in range(ndblk):
                        d0 = dblk * DBLK
                        dsl = slice(d0, d0 + DBLK)
                        ewT, eb64 = enc_tiles
                        if dblk + 1 < ndblk:
                            enc_tiles = _encode_prep(nc, encp, ebps, ew_d,
                                                     ew16_d, eb_d, dblk + 1,
                                                     nct, c)
                        for bt in range(nb):
                            half, hb = divmod(bt, nh)
                            xcT = xcT_h[half]
                            bsl = slice(hb * P, (hb + 1) * P)
                            ph = encps.tile([P, DBLK], F32, tag="hps")
                            for ct in range(nct):
                                nc.tensor.matmul(
                                    ph, xcT[:, ct, bsl], ewT[:, ct, :],
                                    start=(ct == 0), stop=False)
                                nc.tensor.matmul(
                                    ph, xcT[:, ct, bsl], ewT[:, nct + ct, :],
                                    start=False, stop=False)
                                nc.tensor.matmul(
                                    ph, xcT[:, nct + ct, bsl], ewT[:, ct, :],
                                    start=False, stop=(ct == nct - 1))
                            nc.vector.tensor_add(ph, ph, eb64)
                            hsb = encs.tile([P, DBLK], F32, tag="hsb")
                            nc.scalar.activation(hsb, ph, AF.Copy,
                                                 scale=1.0 / WSCALE)
                            nc.sync.dma_start(
                                out=h_d[bt * P:(bt + 1) * P, dsl], in_=hsb)
                            for ch in range(DBLK // CHUNK):
                                ci = (d0 // CHUNK) + ch
                                nc.vector.max(
                                    out=cand[bt][:, ci * 8:(ci + 1) * 8],
                                    in_=hsb[:, ch * CHUNK:(ch + 1) * CHUNK])

                # threshold: k-th largest per row from candidates
                with tc.tile_pool(name="ph2", bufs=2) as ph2:
                    rounds = (k + 7) // 8
                    for bt in range(nb):
                        scr = ph2.tile([P, 8], F32, tag="scr")
                        for rnd in range(rounds):
                            nc.vector.max(out=scr, in_=cand[bt])
                            if rnd < rounds - 1:
                                nc.vector.match_replace(
                                    out=cand[bt], in_to_replace=scr,
                                    in_values=cand[bt], imm_value=NEG)
                        pos = (k - 1) % 8
                        nc.vector.tensor_scalar_max(
                            t_sb[:, bt:bt + 1], scr[:, pos:pos + 1], 1e-30)

            # ---- decode ----
            ndd = DBLK2 // P
            cb_w = 512
            ncb = c // cb_w
            with (
                tc.tile_pool(name="outp", bufs=1) as outp,
                tc.tile_pool(name="dec", bufs=2) as decp,
                tc.tile_pool(name="decl", bufs=3) as decl,
                tc.tile_pool(name="decs", bufs=1) as decs,
                tc.tile_pool(name="decps", bufs=4, space="PSUM") as decps,
                tc.tile_pool(name="trps", bufs=2, space="PSUM") as trps,
            ):
                out_acc = [outp.tile([P, c], F32, tag=f"oacc{bt}",
                                     name=f"oacc{bt}") for bt in range(nb)]

                def dec_load(d2):
                    d0 = d2 * DBLK2
                    dwn16 = decl.tile([P, nct, DBLK2], F16, tag="dwn16")
                    nc.gpsimd.dma_start(
                        out=dwn16,
                        in_=dw_d[:, d0:d0 + DBLK2].rearrange(
                            "(a p) d -> p a d", p=P))
                    hm = []
                    for bt in range(nb):
                        bsl = slice(bt * P, (bt + 1) * P)
                        hblk = decs.tile([P, DBLK2], F32, tag="hldb",
                                         bufs=3)
                        nc.sync.dma_start(out=hblk,
                                          in_=h_d[bsl, d0:d0 + DBLK2])
                        msk = decs.tile([P, DBLK2], F16, tag="msk",
                                        bufs=3)
                        nc.vector.tensor_tensor(
                            out=msk, in0=hblk,
                            in1=t_sb[:, bt:bt + 1].broadcast_to([P, DBLK2]),
                            op=ALU.is_ge)
                        hm16 = decs.tile([P, DBLK2], F16, tag="hm16",
                                         bufs=16)
                        nc.gpsimd.tensor_mul(hm16, hblk, msk)
                        hm.append(hm16)
                    return dwn16, hm

                def dec_transpose(tiles):
                    dwn16, hm = tiles
                    dwT = decp.tile([P, ndd, c], F16, tag="dwT")
                    for a in range(nct):
                        pw = trps.tile([P, DBLK2], F16, tag="dwtr")
                        for dd in range(ndd):
                            nc.tensor.transpose(
                                pw[:, dd * P:(dd + 1) * P],
                                dwn16[:, a, dd * P:(dd + 1) * P], ident16)
                        nc.scalar.activation(
                            dwT[:, :, a * P:(a + 1) * P],
                            pw.rearrange("p (u q) -> p u q", q=P), AF.Copy)
                    hmT = decp.tile([P, ndd, b], F16, tag="hmT")
                    for bt in range(nb):
                        bsl = slice(bt * P, (bt + 1) * P)
                        pw = trps.tile([P, DBLK2], F16, tag="hmtr")
                        for dd in range(ndd):
                            nc.tensor.transpose(
                                pw[:, dd * P:(dd + 1) * P],
                                hm[bt][:, dd * P:(dd + 1) * P], ident16)
                        nc.scalar.activation(
                            hmT[:, :, bsl],
                            pw.rearrange("p (u q) -> p u q", q=P), AF.Copy)
                    return dwT, hmT

                loads = {0: dec_load(0)}
                if nd2 > 1:
                    loads[1] = dec_load(1)
                for d2 in range(nd2):
                    dwT, hmT = dec_transpose(loads.pop(d2))
                    if d2 + 2 < nd2:
                        loads[d2 + 2] = dec_load(d2 + 2)
                    first = (d2 == 0)
                    for bt in range(nb):
                        bsl = slice(bt * P, (bt + 1) * P)
                        for cb in range(ncb):
                            cs = slice(cb * cb_w, (cb + 1) * cb_w)
                            po = decps.tile([P, cb_w], F32, tag="ops")
                            if first:
                                nc.tensor.matmul(po, ones16, db_hi[:, cs],
                                                 start=True, stop=False)
                                nc.tensor.matmul(po, ones16, db_lo[:, cs],
                                                 start=False, stop=False)
                            for dd in range(ndd):
                                nc.tensor.matmul(
                                    po, hmT[:, dd, bsl], dwT[:, dd, cs],
                                    start=(dd == 0 and not first),
                                    stop=(dd == ndd - 1))
                            if first:
                                nc.scalar.activation(out_acc[bt][:, cs], po,
                                                     AF.Copy)
                            else:
                                nc.vector.tensor_add(out_acc[bt][:, cs],
                                                     out_acc[bt][:, cs], po)

                for bt in range(nb):
                    nc.sync.dma_start(out=out_d[bt * P:(bt + 1) * P, :],
                                      in_=out_acc[bt])
    return nc


def run(x, encoder_w, encoder_b, decoder_w, decoder_b, k, trace=False):
    x = np.ascontiguousarray(np.asarray(x, dtype=np.float32))
    encoder_w = np.asarray(encoder_w, dtype=np.float32)
    encoder_b = np.asarray(encoder_b, dtype=np.float32)
    decoder_w = np.asarray(decoder_w, dtype=np.float32)
    decoder_b = np.asarray(decoder_b, dtype=np.float32)
    k = int(k)
    b, r, c = x.shape
    d = encoder_w.shape[1]
    assert (b, r, c, d) == (B, R, C, D), (b, r, c, d)

    nc = build(k)
    if not nc.is_finalized():
        nc.finalize()
    in_maps = []
    for i in range(r):
        in_maps.append({
            "x": np.ascontiguousarray(x[:, i, :]),
            "encoder_w": np.ascontiguousarray(encoder_w[i]),
            "encoder_b": np.ascontiguousarray(encoder_b[i]),
            "decoder_w": np.ascontiguousarray(decoder_w[i]),
            "decoder_b": np.ascontiguousarray(decoder_b[i]),
        })
    res = run_bass_kernel_spmd(nc, in_maps, core_ids=list(range(N_CORES)),
                               trace=trace)
    out = np.empty((b, r, c), dtype=np.float32)
    for i in range(r):
        out[:, i, :] = res.results[i]["out"]
    return out, res


def kernel(x, encoder_w, encoder_b, decoder_w, decoder_b, k):
    out, _ = run(x, encoder_w, encoder_b, decoder_w, decoder_b, k)
    return out


# revision 3
# speedup vs baseline: 1.0610x; 1.0610x over previous
"""DenseTopKSAE kernel v4 for Trainium2 (8 NeuronCores, Bass/Tile).

Sharding: expert-parallel over R -- core r owns SAE r. No collectives.

Structure (per core):
  X: x loaded, db subtracted (ones-matmul broadcast), fp16 hi/lo split,
     staged to DRAM per b-half, dense xbar transpose-load -> xcT halves.
     The encoder-weight chain for block 0 is issued first so both
     startup chains overlap.
  E: per 512-d block (software-pipelined one block ahead): ew fp32 load,
     64-scaled fp16 hi/lo split (ACT + one fused DVE op), staged to DRAM
     [d, hi|lo], ONE xbar transpose-load -> ewT [128c, 16, 512]; eb
     broadcast via K=1 ones-matmuls; 24 fp16 MMs per b-tile (3-term
     split, fp32-exact); +eb, un-scale, h -> DRAM fp32; DVE max8
     candidates per 256-chunk.
  T: max8 + match_replace rounds on candidates -> k-th largest t per row.
  D: per 1024-d block, loads prefetched two blocks ahead: dw loaded with
     SWDGE cast-DMA directly to fp16; h reloaded; mask = (h>=t)*h as
     DVE is_ge (stride-0 broadcast of t) + gpsimd multiply; dw/hm
     PE-transposed (fp16, cheap) right before the block's MMs; 8 MMs per
     (b-tile, c-half) accumulate 1024 d; db bias via ones-MMs on block
     0; output accumulated in SBUF (DVE adds), stored at the end.
"""

import numpy as np

import concourse.bass as bass
import concourse.mybir as mybir
import concourse.tile as tile
from concourse import bacc
from concourse.bass_utils import run_bass_kernel_spmd

F32 = mybir.dt.float32
F16 = mybir.dt.float16
F8 = mybir.dt.float8e4
PM = mybir.MatmulPerfMode
AF = mybir.ActivationFunctionType
ALU = mybir.AluOpType
P = 128
NEG = -3.0e38
WSCALE = 64.0
SLO = 512.0   # fp8 scale for the lo cross-term operands

B, R, C, D = 1024, 8, 1024, 16384
N_CORES = 8

DBLK = 512      # encode d-block
DBLK2 = 1024    # decode d-block
CHUNK = 256     # candidate chunk (top-8 per chunk must cover top-k)


def _mk_identity(nc, ident, fill):
    nc.gpsimd.memset(ident, 0.0)
    nc.gpsimd.affine_select(
        out=ident, in_=ident, compare_op=ALU.not_equal, fill=fill,
        base=0, pattern=[[-1, ident.shape[0]]], channel_multiplier=1,
    )


def _encode_prep(nc, encp, ebps, ew_d, ew16_d, eb_d, dblk, nct, c):
    """Stage + transpose-load one 512-d block of encoder weight + bias."""
    na = DBLK // P
    d0 = dblk * DBLK
    dsl = slice(d0, d0 + DBLK)
    ewn = encp.tile([P, na, c], F32, tag="ewn", bufs=1)
    nc.sync.dma_start(
        out=ewn, in_=ew_d[dsl, :].rearrange("(a p) c -> p a c", p=P))
    wh = encp.tile([P, na, c], F16, tag="wh")
    nc.scalar.activation(wh, ewn, AF.Copy, scale=WSCALE)
    wl = encp.tile([P, na, c], F16, tag="wl")
    nc.vector.scalar_tensor_tensor(
        out=wl, in0=ewn, scalar=WSCALE, in1=wh,
        op0=ALU.mult, op1=ALU.subtract)
    nc.sync.dma_start(
        out=ew16_d[dsl, 0:c].rearrange("(a p) c -> p a c", p=P), in_=wh)
    nc.sync.dma_start(
        out=ew16_d[dsl, c:2 * c].rearrange("(a p) c -> p a c", p=P), in_=wl)
    ewT = encp.tile([P, 2 * nct, DBLK], F16, tag="ewT")
    nc.sync.dma_start(out=ewT, in_=ew16_d[dsl, :], transpose=True)
    # fp8 pairs for the DoubleRow cross-term pass: [c, ct, (SLO*w_lo, w_hi)]
    ewT8 = encp.tile([P, nct, 2, DBLK], F8, tag="ewT8")
    nc.scalar.activation(ewT8[:, :, 0, :], ewT[:, nct:, :], AF.Copy,
                         scale=SLO)
    nc.scalar.activation(ewT8[:, :, 1, :], ewT[:, 0:nct, :], AF.Copy)
    ebs = encp.tile([1, DBLK], F32, tag="ebs", bufs=1)
    nc.sync.dma_start(out=ebs,
                      in_=eb_d[dsl].rearrange("(a n) -> a n", a=1))
    ebhl = encp.tile([1, 2, DBLK], F16, tag="ebhl", bufs=1)
    nc.vector.tensor_copy(ebhl[:, 0, :], ebs)
    nc.vector.tensor_sub(ebhl[:, 1, :], ebs, ebhl[:, 0, :])
    pe_b = ebps.tile([P, DBLK], F32, tag="ebps")
    nc.tensor.matmul(pe_b, ones_g[0], ebhl[:, 0, :], start=True, stop=False)
    nc.tensor.matmul(pe_b, ones_g[0], ebhl[:, 1, :], start=False, stop=True)
    eb64 = encp.tile([P, DBLK], F32, tag="eb64")
    nc.scalar.activation(eb64, pe_b, AF.Copy, scale=WSCALE)
    return ewT, ewT8, eb64


ones_g = [None]


def build(k, b=B, c=C, d=D):
    nb, nct = b // P, c // P
    ndblk = d // DBLK
    nd2 = d // DBLK2
    nch = d // CHUNK
    nh = nb // 2

    nc = bacc.Bacc("TRN2", target_bir_lowering=False, debug=False,
                   num_devices=N_CORES)
    x_d = nc.declare_dram_parameter("x", [b, c], F32, isOutput=False)
    ew_d = nc.declare_dram_parameter("encoder_w", [d, c], F32, isOutput=False)
    eb_d = nc.declare_dram_parameter("encoder_b", [d], F32, isOutput=False)
    dw_d = nc.declare_dram_parameter("decoder_w", [c, d], F32, isOutput=False)
    db_d = nc.declare_dram_parameter("decoder_b", [c], F32, isOutput=False)
    out_d = nc.declare_dram_parameter("out", [b, c], F32, isOutput=True)
    h_d = nc.dram_tensor("h_scratch", [b, d], F32)
    ew16_d = nc.dram_tensor("ew16_scratch", [d, 2 * c], F16)
    x16_h = [nc.dram_tensor(f"x16_scratch{i}", [b // 2, 2 * c], F16)
             for i in range(2)]

    with tile.TileContext(nc) as tc:
        with tc.tile_pool(name="persist", bufs=1) as pp:
            ones16 = pp.tile([1, P], F16, tag="ones16")
            nc.vector.memset(ones16, 1.0)
            ones_g[0] = ones16
            ident16 = pp.tile([P, P], F16, tag="ident16")
            _mk_identity(nc, ident16, 1.0)

            db_hi = pp.tile([1, c], F16, tag="dbhi")
            db_lo = pp.tile([1, c], F16, tag="dblo")
            dbu_hi = pp.tile([1, c], F16, tag="dbuhi")
            dbu_lo = pp.tile([1, c], F16, tag="dbulo")
            with tc.tile_pool(name="bprep", bufs=1) as bp:
                db_row = bp.tile([1, c], F32, tag="dbrow")
                nc.sync.dma_start(out=db_row,
                                  in_=db_d.rearrange("(a n) -> a n", a=1))
                db64 = bp.tile([1, c], F32, tag="db64")
                nc.scalar.activation(db64, db_row, AF.Copy, scale=WSCALE)
                nc.vector.tensor_copy(db_hi, db64)
                nc.vector.tensor_sub(db_lo, db64, db_hi)
                nc.vector.tensor_copy(dbu_hi, db_row)
                nc.vector.tensor_sub(dbu_lo, db_row, dbu_hi)

            t_sb = pp.tile([P, nb], F32, tag="tsb")

            with tc.tile_pool(name="candp", bufs=1) as cp:
                cand = [cp.tile([P, nch * 8], F32, tag=f"cand{bt}",
                                name=f"cand{bt}") for bt in range(nb)]
                with (
                    tc.tile_pool(name="xcpool", bufs=1) as xcp,
                    tc.tile_pool(name="enc", bufs=2) as encp,
                    tc.tile_pool(name="encs", bufs=3) as encs,
                    tc.tile_pool(name="encps", bufs=3, space="PSUM") as encps,
                    tc.tile_pool(name="encxps", bufs=3, space="PSUM") as encxps,
                    tc.tile_pool(name="ebps", bufs=1, space="PSUM") as ebps,
                ):
                    xcT_h = [xcp.tile([P, nct, b // 2], F16,
                                      tag=f"xcT{i}", name=f"xcT{i}")
                             for i in range(2)]
                    xcT8_h = [xcp.tile([P, nct, 2, b // 2], F8,
                                       tag=f"xcT8{i}", name=f"xcT8{i}")
                              for i in range(2)]
                    # encoder block 0 chain first (overlaps the x phase)
                    enc_tiles = _encode_prep(nc, encp, ebps, ew_d, ew16_d,
                                             eb_d, 0, nct, c)
                    # db broadcast for the x subtraction
                    xph = tc.tile_pool(name="ph0", bufs=2)
                    ph0 = xph.__enter__()
                    db_bc = ph0.tile([P, c], F32, tag="dbbc", bufs=1)
                    for ci in range(c // 512):
                        csl = slice(ci * 512, (ci + 1) * 512)
                        pb = ebps.tile([P, 512], F32, tag="ebps")
                        nc.tensor.matmul(pb, ones16, dbu_hi[:, csl],
                                         start=True, stop=False)
                        nc.tensor.matmul(pb, ones16, dbu_lo[:, csl],
                                         start=False, stop=True)
                        nc.scalar.activation(db_bc[:, csl], pb, AF.Copy)
                    # x phase: split + stage per b-half
                    for bt in range(nb):
                        xt = ph0.tile([P, c], F32, tag="xstage", bufs=2)
                        nc.sync.dma_start(out=xt,
                                          in_=x_d[bt * P:(bt + 1) * P, :])
                        nc.vector.tensor_sub(xt, xt, db_bc)
                        xh = ph0.tile([P, c], F16, tag="xh")
                        nc.scalar.activation(xh, xt, AF.Copy)
                        xl = ph0.tile([P, c], F16, tag="xl")
                        nc.vector.tensor_sub(xl, xt, xh)
                        half, hb = divmod(bt, nh)
                        hsl = slice(hb * P, (hb + 1) * P)
                        nc.sync.dma_start(out=x16_h[half][hsl, 0:c], in_=xh)
                        nc.sync.dma_start(out=x16_h[half][hsl, c:2 * c],
                                          in_=xl)
                        if hb == nh - 1:
                            nc.sync.dma_start(out=xcT_h[half],
                                              in_=x16_h[half][:, 0:c],
                                              transpose=True)
                            xlT = ph0.tile([P, nct, b // 2], F16, tag="xlT",
                                           bufs=1)
                            nc.sync.dma_start(out=xlT,
                                              in_=x16_h[half][:, c:2 * c],
                                              transpose=True)
                            nc.scalar.activation(
                                xcT8_h[half][:, :, 0, :], xcT_h[half],
                                AF.Copy)
                            nc.scalar.activation(
                                xcT8_h[half][:, :, 1, :], xlT,
                                AF.Copy, scale=SLO)
                        if bt == nb - 1:
                            xph.__exit__(None, None, None)
                    # encode main loop, one-block software pipeline
                    for dblk in range(ndblk):
                        d0 = dblk * DBLK
                        dsl = slice(d0, d0 + DBLK)
                        ewT, ewT8, eb64 = enc_tiles
                        if dblk + 1 < ndblk:
                            enc_tiles = _encode_prep(nc, encp, ebps, ew_d,
                                                     ew16_d, eb_d, dblk + 1,
                                                     nct, c)
                        for bt in range(nb):
                            half, hb = divmod(bt, nh)
                            xcT = xcT_h[half]
                            xcT8 = xcT8_h[half]
                            bsl = slice(hb * P, (hb + 1) * P)
                            ph = encps.tile([P, DBLK], F32, tag="hps")
                            for ct in range(nct):
                                nc.tensor.matmul(
                                    ph, xcT[:, ct, bsl], ewT[:, ct, :],
                                    start=(ct == 0), stop=(ct == nct - 1))
                            px = encxps.tile([P, DBLK], F32, tag="xps")
                            for ct in range(nct):
                                nc.tensor.matmul(
                                    px, xcT8[:, ct, :, bsl],
                                    ewT8[:, ct, :, :],
                                    start=(ct == 0), stop=(ct == nct - 1),
                                    perf_mode=PM.DoubleRow)
                            hsb = encs.tile([P, DBLK], F32, tag="hsb")
                            nc.vector.tensor_add(hsb, ph, eb64)
                            nc.vector.scalar_tensor_tensor(
                                out=hsb, in0=px, scalar=1.0 / SLO, in1=hsb,
                                op0=ALU.mult, op1=ALU.add)
                            nc.sync.dma_start(
                                out=h_d[bt * P:(bt + 1) * P, dsl], in_=hsb)
                            for ch in range(DBLK // CHUNK):
                                ci = (d0 // CHUNK) + ch
                                nc.vector.max(
                                    out=cand[bt][:, ci * 8:(ci + 1) * 8],
                                    in_=hsb[:, ch * CHUNK:(ch + 1) * CHUNK])

                # threshold: k-th largest per row from candidates
                with tc.tile_pool(name="ph2", bufs=2) as ph2:
                    rounds = (k + 7) // 8
                    for bt in range(nb):
                        scr = ph2.tile([P, 8], F32, tag="scr")
                        for rnd in range(rounds):
                            nc.vector.max(out=scr, in_=cand[bt])
                            if rnd < rounds - 1:
                                nc.vector.match_replace(
                                    out=cand[bt], in_to_replace=scr,
                                    in_values=cand[bt], imm_value=NEG)
                        pos = (k - 1) % 8
                        nc.vector.tensor_scalar_max(
                            t_sb[:, bt:bt + 1], scr[:, pos:pos + 1], 1e-30)

            # ---- decode ----
            ndd = DBLK2 // P
            cb_w = 512
            ncb = c // cb_w
            with (
                tc.tile_pool(name="outp", bufs=1) as outp,
                tc.tile_pool(name="dec", bufs=2) as decp,
                tc.tile_pool(name="decl", bufs=3) as decl,
                tc.tile_pool(name="decs", bufs=1) as decs,
                tc.tile_pool(name="decps", bufs=4, space="PSUM") as decps,
                tc.tile_pool(name="trps", bufs=2, space="PSUM") as trps,
            ):
                out_acc = [outp.tile([P, c], F32, tag=f"oacc{bt}",
                                     name=f"oacc{bt}") for bt in range(nb)]

                def dec_load(d2):
                    d0 = d2 * DBLK2
                    dwn16 = decl.tile([P, nct, DBLK2], F16, tag="dwn16")
                    nc.gpsimd.dma_start(
                        out=dwn16,
                        in_=dw_d[:, d0:d0 + DBLK2].rearrange(
                            "(a p) d -> p a d", p=P))
                    hm = []
                    for bt in range(nb):
                        bsl = slice(bt * P, (bt + 1) * P)
                        hblk = decs.tile([P, DBLK2], F32, tag="hldb",
                                         bufs=3)
                        nc.sync.dma_start(out=hblk,
                                          in_=h_d[bsl, d0:d0 + DBLK2])
                        msk = decs.tile([P, DBLK2], F16, tag="msk",
                                        bufs=3)
                        nc.vector.tensor_tensor(
                            out=msk, in0=hblk,
                            in1=t_sb[:, bt:bt + 1].broadcast_to([P, DBLK2]),
                            op=ALU.is_ge)
                        hm16 = decs.tile([P, DBLK2], F16, tag="hm16",
                                         bufs=16)
                        nc.gpsimd.tensor_mul(hm16, hblk, msk)
                        hm.append(hm16)
                    return dwn16, hm

                def dec_transpose(tiles):
                    dwn16, hm = tiles
                    dwT = decp.tile([P, ndd, c], F16, tag="dwT")
                    for a in range(nct):
                        pw = trps.tile([P, DBLK2], F16, tag="dwtr")
                        for dd in range(ndd):
                            nc.tensor.transpose(
                                pw[:, dd * P:(dd + 1) * P],
                                dwn16[:, a, dd * P:(dd + 1) * P], ident16)
                        nc.scalar.activation(
                            dwT[:, :, a * P:(a + 1) * P],
                            pw.rearrange("p (u q) -> p u q", q=P), AF.Copy)
                    hmT = decp.tile([P, ndd, b], F16, tag="hmT")
                    for bt in range(nb):
                        bsl = slice(bt * P, (bt + 1) * P)
                        pw = trps.tile([P, DBLK2], F16, tag="hmtr")
                        for dd in range(ndd):
                            nc.tensor.transpose(
                                pw[:, dd * P:(dd + 1) * P],
                                hm[bt][:, dd * P:(dd + 1) * P], ident16)
                        nc.scalar.activation(
                            hmT[:, :, bsl],
                            pw.rearrange("p (u q) -> p u q", q=P), AF.Copy)
                    return dwT, hmT

                loads = {0: dec_load(0)}
                if nd2 > 1:
                    loads[1] = dec_load(1)
                for d2 in range(nd2):
                    dwT, hmT = dec_transpose(loads.pop(d2))
                    if d2 + 2 < nd2:
                        loads[d2 + 2] = dec_load(d2 + 2)
                    first = (d2 == 0)
                    for bt in range(nb):
                        bsl = slice(bt * P, (bt + 1) * P)
                        for cb in range(ncb):
                            cs = slice(cb * cb_w, (cb + 1) * cb_w)
                            po = decps.tile([P, cb_w], F32, tag="ops")
                            if first:
                                nc.tensor.matmul(po, ones16, db_hi[:, cs],
                                                 start=True, stop=False)
                                nc.tensor.matmul(po, ones16, db_lo[:, cs],
                                                 start=False, stop=False)
                            for dd in range(ndd):
                                nc.tensor.matmul(
                                    po, hmT[:, dd, bsl], dwT[:, dd, cs],
                                    start=(dd == 0 and not first),
                                    stop=(dd == ndd - 1))
                            if first:
                                nc.scalar.activation(out_acc[bt][:, cs], po,
                                                     AF.Copy,
                                                     scale=1.0 / WSCALE)
                            else:
                                nc.vector.scalar_tensor_tensor(
                                    out=out_acc[bt][:, cs], in0=po,
                                    scalar=1.0 / WSCALE,
                                    in1=out_acc[bt][:, cs],
                                    op0=ALU.mult, op1=ALU.add)

                for bt in range(nb):
                    nc.sync.dma_start(out=out_d[bt * P:(bt + 1) * P, :],
                                      in_=out_acc[bt])
    return nc


def run(x, encoder_w, encoder_b, decoder_w, decoder_b, k, trace=False):
    x = np.ascontiguousarray(np.asarray(x, dtype=np.float32))
    encoder_w = np.asarray(encoder_w, dtype=np.float32)
    encoder_b = np.asarray(encoder_b, dtype=np.float32)
    decoder_w = np.asarray(decoder_w, dtype=np.float32)
    decoder_b = np.asarray(decoder_b, dtype=np.float32)
    k = int(k)
    b, r, c = x.shape
    d = encoder_w.shape[1]
    assert (b, r, c, d) == (B, R, C, D), (b, r, c, d)

    nc = build(k)
    if not nc.is_finalized():
        nc.finalize()
    in_maps = []
    for i in range(r):
        in_maps.append({
            "x": np.ascontiguousarray(x[:, i, :]),
            "encoder_w": np.ascontiguousarray(encoder_w[i]),
            "encoder_b": np.ascontiguousarray(encoder_b[i]),
            "decoder_w": np.ascontiguousarray(decoder_w[i]),
            "decoder_b": np.ascontiguousarray(decoder_b[i]),
        })
    res = run_bass_kernel_spmd(nc, in_maps, core_ids=list(range(N_CORES)),
                               trace=trace)
    out = np.empty((b, r, c), dtype=np.float32)
    for i in range(r):
        out[:, i, :] = res.results[i]["out"]
    return out, res


def kernel(x, encoder_w, encoder_b, decoder_w, decoder_b, k):
    out, _ = run(x, encoder_w, encoder_b, decoder_w, decoder_b, k)
    return out
